# revision 15
# baseline (speedup 1.0000x reference)
"""Trainium2 Bass kernel for nn_ModelNew_3556232921999.

Pipeline: ConvTranspose3d(16->32, k=3, s=2, p=1, op=1) -> MaxPool3d(2)
          -> softmax(ch) -> subtract -> swish -> max(ch)

Algebraic structure:
  * convT(stride 2) + maxpool(2) => pooled[c, m] = max over 8 parity classes,
    each a {0,1}^3-offset tap-conv of x. One matmul per 128 positions:
      lhsT = x-stack block [K=128=(od,oh,ow,cin), M=128 positions] (stationary)
      rhs  = W            [K=128, N=256=(c,parity)]                (moving)
    PSUM columns ordered (c outer, parity inner) so the parity-max is an
    innermost-axis reduce.
  * max_c silu(v_c) = max(silu(max_c v), silu(min_c v)) (silu quasiconvex).

V2 (from baseline trace: Vector 365us busy / 365us wait was the bottleneck):
  * stage-1 parity-max split across THREE engines working directly on PSUM:
      DVE  tensor_reduce(X)  channels [0, C0)
      Pool pairwise max tree channels [C0, C1)
      Act  copy-evacuate     channels [C1, 32) + DVE bf16 4x fold
  * softmax divide via reciprocal + DMA stride-0 broadcast (DMA is idle)
  * all epilogue ops batched per 8-block group (1024 positions)
  * silu tail once per batch-slice at the very end (2 ACT table swaps total)

Sharding: data-parallel over batch B=16 -> 2 per core x 8 cores.
"""

import os
import sys

sys.path.insert(0, "/opt/trn_rl_repo")

import numpy as np
import ml_dtypes

# ---------------------------------------------------------------- constants
IN_C, OUT_C, K, STRIDE, PAD, OUT_PAD = 16, 32, 3, 2, 1, 1
B, D, H, W = 16, 16, 64, 64
N_CORES = 8
B_PER_CORE = B // N_CORES  # 2

PLANE = H * W            # 4096 positions per (b, d) plane
BLK = 128                # positions per matmul block
BLKS_PER_PLANE = PLANE // BLK      # 32
GRP = 8                  # matmul blocks per psum group (1024 positions)
GRPS_PER_PLANE = BLKS_PER_PLANE // GRP  # 4

# stage-1 channel split: [0,C0) DVE reduce from PSUM, [C0,32) Act evacuate
# then DVE folds the evacuation (bf16 4x). Pool/GpSimd can neither access
# PSUM nor execute max ops on this target, so it only gets the arithmetic
# stages (bias add, softmax multiply, subtract).
C0 = 12

X_NP_DT = ml_dtypes.bfloat16
W_NP_DT = ml_dtypes.bfloat16

_COMPILED = {}
LAST_EXEC_NS = None
LAST_RESULTS = None


def _tap(o, p):
    """Kernel tap index used by parity class p at window offset o, or None."""
    if p == 0:
        return 1 if o == 0 else None
    return 2 if o == 0 else 0


def build_wrhs(weight):
    """[128 rows=(od,oh,ow,cin), 256 cols=(c, pd,ph,pw)] conv matrix."""
    wr = np.zeros((2, 2, 2, IN_C, OUT_C, 2, 2, 2), dtype=np.float32)
    for od in range(2):
        for oh in range(2):
            for ow in range(2):
                for pd in range(2):
                    kd = _tap(od, pd)
                    if kd is None:
                        continue
                    for ph in range(2):
                        kh = _tap(oh, ph)
                        if kh is None:
                            continue
                        for pw in range(2):
                            kw = _tap(ow, pw)
                            if kw is None:
                                continue
                            # weight: [cin, cout, kd, kh, kw]
                            wr[od, oh, ow, :, :, pd, ph, pw] = weight[:, :, kd, kh, kw]
    return wr.reshape(128, 256)


def build_xstack(x):
    """[B, D, 128 rows=(od,oh,ow,cin), PLANE] shifted/padded copies of x."""
    xp = np.zeros((B, IN_C, D + 1, H + 1, W + 1), dtype=np.float32)
    xp[:, :, :D, :H, :W] = x
    S = np.empty((B, D, 2, 2, 2, IN_C, H, W), dtype=X_NP_DT)
    for od in range(2):
        for oh in range(2):
            for ow in range(2):
                sl = xp[:, :, od:od + D, oh:oh + H, ow:ow + W]
                S[:, :, od, oh, ow] = sl.transpose(0, 2, 1, 3, 4).astype(X_NP_DT)
    return S.reshape(B, D, 128, PLANE)


def build_kernel():
    from concourse import bass, bacc, mybir, tile

    f32 = mybir.dt.float32
    bf16 = mybir.dt.bfloat16
    x_dt = bf16
    w_dt = bf16
    Alu = mybir.AluOpType
    Act = mybir.ActivationFunctionType
    Ax = mybir.AxisListType

    nc = bacc.Bacc("TRN2", target_bir_lowering=False, debug=False,
                   num_devices=N_CORES)

    xs_h = nc.declare_dram_parameter("xs", [B_PER_CORE, D, 128, PLANE], x_dt,
                                     isOutput=False)
    wr_h = nc.declare_dram_parameter("wr", [128, 256], w_dt, isOutput=False)
    bias_h = nc.declare_dram_parameter("biasrep", [128, 256], bf16,
                                       isOutput=False)
    sub_h = nc.declare_dram_parameter("subrep", [128, 256], bf16,
                                      isOutput=False)
    y_h = nc.declare_dram_parameter("y", [B_PER_CORE, D, PLANE], f32,
                                    isOutput=True)

    NACT = OUT_C - C0    # channels evacuated by Act, folded by Pool

    from contextlib import ExitStack

    with tile.TileContext(nc) as tc:
        with ExitStack() as stack:
            pool_specs = dict(
                const=1, xslab=3, g1=2, g2=2, ev=2, pooled=2, pb=2, exp=2,
                z=2, r=2, rrep=2, m=2, v=2, ext=1, sil=1, ost=1)
            pools = {
                name: stack.enter_context(tc.tile_pool(name=name, bufs=n))
                for name, n in pool_specs.items()}
            pools["psum"] = stack.enter_context(
                tc.tile_pool(name="psum", bufs=2, space="PSUM"))
            constp, xpool, psump = (pools[k] for k in ("const", "xslab", "psum"))
            g1p, g2p, evp, plp, pbp = (
                pools[k] for k in ("g1", "g2", "ev", "pooled", "pb"))
            ep, zp, rp, rrepp, mp, vp = (
                pools[k] for k in ("exp", "z", "r", "rrep", "m", "v"))
            extp, silp, ostp = (pools[k] for k in ("ext", "sil", "ost"))
            wr = constp.tile([128, 256], w_dt)
            nc.sync.dma_start(wr[:], wr_h[:, :])
            biasrep = constp.tile([128, GRP, OUT_C], bf16)
            nc.sync.dma_start(
                biasrep[:].rearrange("p a b -> p (a b)"), bias_h[:, :])
            subrep = constp.tile([128, GRP, OUT_C], bf16)
            nc.sync.dma_start(
                subrep[:].rearrange("p a b -> p (a b)"), sub_h[:, :])

            exts = []
            for b in range(B_PER_CORE):
                # vmax/vmin staging: [128, 2(ismin), D*GRPS*GRP = 512]
                ext = extp.tile([128, 2, D * GRPS_PER_PLANE * GRP], f32,
                                tag=f"ext{b}")
                exts.append(ext)
                for d in range(D):
                    slab = xpool.tile([128, PLANE], x_dt, tag="slab")
                    nc.sync.dma_start(slab[:], xs_h[b, d])
                    for g in range(GRPS_PER_PLANE):
                        psum = psump.tile([128, GRP, OUT_C, 8], f32, tag="ps")
                        for k in range(GRP):
                            blk = (g * GRP + k) * BLK
                            nc.tensor.matmul(
                                psum[:, k].rearrange("p c q -> p (c q)"),
                                slab[:, blk:blk + BLK],
                                wr[:],
                                start=True, stop=True,
                            )
                        pooled = plp.tile([128, GRP, OUT_C], bf16, tag="pool")
                        # --- stage 1: parity max 8 -> 1 ---
                        nc.vector.tensor_reduce(
                            pooled[:, :, 0:C0], psum[:, :, 0:C0, :],
                            axis=Ax.X, op=Alu.max)
                        ev = evp.tile([128, GRP, NACT, 8], bf16, tag="ev")
                        nc.scalar.activation(
                            ev[:], psum[:, :, C0:OUT_C, :], Act.Copy)
                        nc.vector.tensor_reduce(
                            pooled[:, :, C0:OUT_C], ev[:], axis=Ax.X,
                            op=Alu.max)
                        # --- stage 2 ---
                        pb = pbp.tile([128, GRP, OUT_C], bf16, tag="pb")
                        nc.gpsimd.tensor_tensor(pb[:], pooled[:], biasrep[:],
                                                Alu.add)
                        E = ep.tile([128, GRP, OUT_C], bf16, tag="E")
                        nc.scalar.activation(E[:], pb[:], Act.Exp)
                        Z = zp.tile([128, GRP], f32, tag="Z")
                        nc.vector.tensor_reduce(Z[:], E[:], axis=Ax.X,
                                                op=Alu.add)
                        R = rp.tile([128, GRP], bf16, tag="R")
                        with nc.allow_low_precision(reason="1/Z bf16 ok"):
                            nc.vector.reciprocal(R[:], Z[:])
                        # DMA-broadcast 1/Z: stride-0 dim must not be the DMA's
                        # fastest-moving dim, so materialize transposed [c, g]
                        RrepT = rrepp.tile([128, OUT_C, GRP], bf16, tag="Rr")
                        nc.sync.dma_start(
                            RrepT[:],
                            R[:].unsqueeze(1).broadcast_to([128, OUT_C, GRP]))
                        m = mp.tile([128, GRP, OUT_C], bf16, tag="m")
                        nc.gpsimd.tensor_tensor(
                            m[:], E[:], RrepT[:].rearrange("p c g -> p g c"),
                            Alu.mult)
                        v = vp.tile([128, GRP, OUT_C], bf16, tag="v")
                        nc.gpsimd.tensor_tensor(v[:], m[:], subrep[:],
                                                Alu.subtract)
                        col = (d * GRPS_PER_PLANE + g) * GRP
                        nc.vector.tensor_reduce(
                            ext[:, 0, col:col + GRP], v[:], axis=Ax.X,
                            op=Alu.max)
                        nc.vector.tensor_reduce(
                            ext[:, 1, col:col + GRP], v[:], axis=Ax.X,
                            op=Alu.min)
            # ---- tail: tiny silu + final pairwise max, both b at once ----
            for b in range(B_PER_CORE):
                ext = exts[b]
                sil = silp.tile([128, 2, 512], f32, tag=f"sil{b}")
                nc.scalar.activation(
                    sil[:].rearrange("p a b -> p (a b)"),
                    ext[:].rearrange("p a b -> p (a b)"), Act.Silu)
                ost = ostp.tile([128, 512], f32, tag=f"ost{b}")
                nc.vector.tensor_tensor(ost[:], sil[:, 0, :], sil[:, 1, :],
                                        Alu.max)
                nc.sync.dma_start(
                    y_h[b].flatten().rearrange(
                        "(dd hg blk p) -> p dd hg blk",
                        dd=D, hg=GRPS_PER_PLANE, blk=GRP, p=BLK),
                    ost[:].rearrange("p (dd hg blk) -> p dd hg blk",
                                     dd=D, hg=GRPS_PER_PLANE, blk=GRP))

    nc.compile()
    return nc


def _get_nc():
    if "nc" not in _COMPILED:
        _COMPILED["nc"] = build_kernel()
    return _COMPILED["nc"]


def kernel(x, weight, bias, subtract):
    from concourse.bass_utils import run_bass_kernel_spmd

    x = np.asarray(x, dtype=np.float32)
    weight = np.asarray(weight, dtype=np.float32)
    bias = np.asarray(bias, dtype=np.float32)
    subtract = np.asarray(subtract, dtype=np.float32)

    nc = _get_nc()

    xs = build_xstack(x)                      # [B, D, 128, PLANE]
    wr = build_wrhs(weight).astype(W_NP_DT)   # [128, 256] cols (c, parity)
    # biasrep/subrep: [128, 256] = (8 groups x 32 ch) pattern, bf16
    biasrep = np.tile(bias[None, None, :], (128, GRP, 1)).reshape(128, 256)
    biasrep = biasrep.astype(ml_dtypes.bfloat16)
    subrep = np.tile(subtract[None, None, :], (128, GRP, 1)).reshape(
        128, 256).astype(ml_dtypes.bfloat16)

    in_maps = []
    for c in range(N_CORES):
        in_maps.append({
            "xs": np.ascontiguousarray(xs[c * B_PER_CORE:(c + 1) * B_PER_CORE]),
            "wr": wr,
            "biasrep": biasrep,
            "subrep": subrep,
        })

    kw = {}
    if os.environ.get("KERNEL_TRACE_DIR"):
        kw["tmpdir"] = os.environ["KERNEL_TRACE_DIR"]
    res = run_bass_kernel_spmd(nc, in_maps, core_ids=list(range(N_CORES)), **kw)
    global LAST_EXEC_NS, LAST_RESULTS
    LAST_EXEC_NS = res.exec_time_ns
    LAST_RESULTS = res
    outs = [res.results[c]["y"].reshape(B_PER_CORE, D, H, W)
            for c in range(N_CORES)]
    return np.concatenate(outs, axis=0)


# revision 16
# speedup vs baseline: 1.5773x; 1.5773x over previous
"""Trainium2 Bass kernel for nn_ModelNew_3556232921999.

Pipeline: ConvTranspose3d(16->32, k=3, s=2, p=1, op=1) -> MaxPool3d(2)
          -> softmax(ch) -> subtract -> swish -> max(ch)

Algebraic structure:
  * convT(stride 2) + maxpool(2) => pooled[c, m] = max over 8 parity classes,
    each a {0,1}^3-offset tap-conv of x. One matmul per 128 positions:
      lhsT = x-stack block [K=128=(od,oh,ow,cin), M=128 positions] (stationary)
      rhs  = W            [K=128, N=256=(c,parity)]                (moving)
    PSUM columns ordered (c outer, parity inner) so the parity-max is an
    innermost-axis reduce.
  * max_c silu(v_c) = max(silu(max_c v), silu(min_c v)) (silu quasiconvex).

V2 (from baseline trace: Vector 365us busy / 365us wait was the bottleneck):
  * stage-1 parity-max split across THREE engines working directly on PSUM:
      DVE  tensor_reduce(X)  channels [0, C0)
      Pool pairwise max tree channels [C0, C1)
      Act  copy-evacuate     channels [C1, 32) + DVE bf16 4x fold
  * softmax divide via reciprocal + DMA stride-0 broadcast (DMA is idle)
  * all epilogue ops batched per 8-block group (1024 positions)
  * silu tail once per batch-slice at the very end (2 ACT table swaps total)

Sharding: data-parallel over batch B=16 -> 2 per core x 8 cores.
"""

import os
import sys

sys.path.insert(0, "/opt/trn_rl_repo")

import numpy as np
import ml_dtypes

# ---------------------------------------------------------------- constants
IN_C, OUT_C, K, STRIDE, PAD, OUT_PAD = 16, 32, 3, 2, 1, 1
B, D, H, W = 16, 16, 64, 64
N_CORES = 8
B_PER_CORE = B // N_CORES  # 2

PLANE = H * W            # 4096 positions per (b, d) plane
BLK = 128                # positions per matmul block
BLKS_PER_PLANE = PLANE // BLK      # 32
GRP = 8                  # matmul blocks per psum group (1024 positions)
GRPS_PER_PLANE = BLKS_PER_PLANE // GRP  # 4

# stage-1 channel split: [0,C0) DVE reduce from PSUM, [C0,32) Act evacuate
# then DVE folds the evacuation (bf16 4x). Pool/GpSimd can neither access
# PSUM nor execute max ops on this target, so it only gets the arithmetic
# stages (bias add, softmax multiply, subtract).
C0 = 12

X_NP_DT = ml_dtypes.bfloat16
W_NP_DT = ml_dtypes.bfloat16

_COMPILED = {}
LAST_EXEC_NS = None
LAST_RESULTS = None


def _tap(o, p):
    """Kernel tap index used by parity class p at window offset o, or None."""
    if p == 0:
        return 1 if o == 0 else None
    return 2 if o == 0 else 0


def build_wrhs(weight):
    """[128 rows=(od,oh,ow,cin), 256 cols=(c, pd,ph,pw)] conv matrix."""
    wr = np.zeros((2, 2, 2, IN_C, OUT_C, 2, 2, 2), dtype=np.float32)
    for od in range(2):
        for oh in range(2):
            for ow in range(2):
                for pd in range(2):
                    kd = _tap(od, pd)
                    if kd is None:
                        continue
                    for ph in range(2):
                        kh = _tap(oh, ph)
                        if kh is None:
                            continue
                        for pw in range(2):
                            kw = _tap(ow, pw)
                            if kw is None:
                                continue
                            # weight: [cin, cout, kd, kh, kw]
                            wr[od, oh, ow, :, :, pd, ph, pw] = weight[:, :, kd, kh, kw]
    return wr.reshape(128, 256)


def build_xstack(x):
    """[B, D, 128 rows=(od,oh,ow,cin), PLANE] shifted/padded copies of x."""
    xp = np.zeros((B, IN_C, D + 1, H + 1, W + 1), dtype=np.float32)
    xp[:, :, :D, :H, :W] = x
    S = np.empty((B, D, 2, 2, 2, IN_C, H, W), dtype=X_NP_DT)
    for od in range(2):
        for oh in range(2):
            for ow in range(2):
                sl = xp[:, :, od:od + D, oh:oh + H, ow:ow + W]
                S[:, :, od, oh, ow] = sl.transpose(0, 2, 1, 3, 4).astype(X_NP_DT)
    return S.reshape(B, D, 128, PLANE)


def build_kernel():
    from concourse import bass, bacc, mybir, tile

    f32 = mybir.dt.float32
    bf16 = mybir.dt.bfloat16
    x_dt = bf16
    w_dt = bf16
    Alu = mybir.AluOpType
    Act = mybir.ActivationFunctionType
    Ax = mybir.AxisListType

    nc = bacc.Bacc("TRN2", target_bir_lowering=False, debug=False,
                   num_devices=N_CORES)

    xs_h = nc.declare_dram_parameter("xs", [B_PER_CORE, D, 128, PLANE], x_dt,
                                     isOutput=False)
    wr_h = nc.declare_dram_parameter("wr", [128, 256], w_dt, isOutput=False)
    bias_h = nc.declare_dram_parameter("biasrep", [128, 256], bf16,
                                       isOutput=False)
    sub_h = nc.declare_dram_parameter("subrep", [128, 256], bf16,
                                      isOutput=False)
    y_h = nc.declare_dram_parameter("y", [B_PER_CORE, D, PLANE], f32,
                                    isOutput=True)

    NACT = OUT_C - C0    # channels evacuated by Act, folded by Pool

    from contextlib import ExitStack

    with tile.TileContext(nc) as tc:
        with ExitStack() as stack:
            pool_specs = dict(
                const=1, xslab=3, g1=2, g2=2, ev=2, pooled=2, pb=2, exp=2,
                z=2, r=2, rrep=2, m=2, v=2, ext=1, sil=1, ost=1)
            pools = {
                name: stack.enter_context(tc.tile_pool(name=name, bufs=n))
                for name, n in pool_specs.items()}
            pools["psum"] = stack.enter_context(
                tc.tile_pool(name="psum", bufs=2, space="PSUM"))
            constp, xpool, psump = (pools[k] for k in ("const", "xslab", "psum"))
            g1p, g2p, evp, plp, pbp = (
                pools[k] for k in ("g1", "g2", "ev", "pooled", "pb"))
            ep, zp, rp, rrepp, mp, vp = (
                pools[k] for k in ("exp", "z", "r", "rrep", "m", "v"))
            extp, silp, ostp = (pools[k] for k in ("ext", "sil", "ost"))
            wr = constp.tile([128, 256], w_dt)
            nc.sync.dma_start(wr[:], wr_h[:, :])
            biasrep = constp.tile([128, GRP, OUT_C], bf16)
            nc.sync.dma_start(
                biasrep[:].rearrange("p a b -> p (a b)"), bias_h[:, :])
            subrep = constp.tile([128, GRP, OUT_C], bf16)
            nc.sync.dma_start(
                subrep[:].rearrange("p a b -> p (a b)"), sub_h[:, :])

            exts = []
            for b in range(B_PER_CORE):
                # vmax/vmin staging: [128, 2(ismin), D*GRPS*GRP = 512]
                ext = extp.tile([128, 2, D * GRPS_PER_PLANE * GRP], f32,
                                tag=f"ext{b}")
                exts.append(ext)
                for d in range(D):
                    slab = xpool.tile([128, PLANE], x_dt, tag="slab")
                    nc.sync.dma_start(slab[:], xs_h[b, d])
                    for g in range(GRPS_PER_PLANE):
                        psum = psump.tile([128, GRP, OUT_C, 8], f32, tag="ps")
                        for k in range(GRP):
                            blk = (g * GRP + k) * BLK
                            nc.tensor.matmul(
                                psum[:, k].rearrange("p c q -> p (c q)"),
                                slab[:, blk:blk + BLK],
                                wr[:],
                                start=True, stop=True,
                            )
                        pooled = plp.tile([128, GRP, OUT_C], bf16, tag="pool")
                        # --- stage 1: parity max 8 -> 1 ---
                        nc.vector.tensor_reduce(
                            pooled[:, :, 0:C0], psum[:, :, 0:C0, :],
                            axis=Ax.X, op=Alu.max)
                        ev = evp.tile([128, GRP, NACT, 8], bf16, tag="ev")
                        nc.scalar.activation(
                            ev[:], psum[:, :, C0:OUT_C, :], Act.Copy)
                        nc.vector.tensor_reduce(
                            pooled[:, :, C0:OUT_C], ev[:], axis=Ax.X,
                            op=Alu.max)
                        # --- stage 2 ---
                        pb = pbp.tile([128, GRP, OUT_C], bf16, tag="pb")
                        nc.gpsimd.tensor_tensor(pb[:], pooled[:], biasrep[:],
                                                Alu.add)
                        E = ep.tile([128, GRP, OUT_C], bf16, tag="E")
                        nc.scalar.activation(E[:], pb[:], Act.Exp)
                        Z = zp.tile([128, GRP], f32, tag="Z")
                        nc.vector.tensor_reduce(Z[:], E[:], axis=Ax.X,
                                                op=Alu.add)
                        R = rp.tile([128, GRP], f32, tag="R")
                        nc.vector.reciprocal(R[:], Z[:])
                        m = mp.tile([128, GRP, OUT_C], bf16, tag="m")
                        nc.gpsimd.tensor_tensor(
                            m[:], E[:],
                            R[:].unsqueeze(2).broadcast_to([128, GRP, OUT_C]),
                            Alu.mult)
                        v = vp.tile([128, GRP, OUT_C], bf16, tag="v")
                        nc.gpsimd.tensor_tensor(v[:], m[:], subrep[:],
                                                Alu.subtract)
                        col = (d * GRPS_PER_PLANE + g) * GRP
                        nc.vector.tensor_reduce(
                            ext[:, 0, col:col + GRP], v[:], axis=Ax.X,
                            op=Alu.max)
                        nc.vector.tensor_reduce(
                            ext[:, 1, col:col + GRP], v[:], axis=Ax.X,
                            op=Alu.min)
            # ---- tail: tiny silu + final pairwise max, both b at once ----
            for b in range(B_PER_CORE):
                ext = exts[b]
                sil = silp.tile([128, 2, 512], f32, tag=f"sil{b}")
                nc.scalar.activation(
                    sil[:].rearrange("p a b -> p (a b)"),
                    ext[:].rearrange("p a b -> p (a b)"), Act.Silu)
                ost = ostp.tile([128, 512], f32, tag=f"ost{b}")
                nc.vector.tensor_tensor(ost[:], sil[:, 0, :], sil[:, 1, :],
                                        Alu.max)
                nc.sync.dma_start(
                    y_h[b].flatten().rearrange(
                        "(dd hg blk p) -> p dd hg blk",
                        dd=D, hg=GRPS_PER_PLANE, blk=GRP, p=BLK),
                    ost[:].rearrange("p (dd hg blk) -> p dd hg blk",
                                     dd=D, hg=GRPS_PER_PLANE, blk=GRP))

    nc.compile()
    return nc


def _get_nc():
    if "nc" not in _COMPILED:
        _COMPILED["nc"] = build_kernel()
    return _COMPILED["nc"]


def kernel(x, weight, bias, subtract):
    from concourse.bass_utils import run_bass_kernel_spmd

    x = np.asarray(x, dtype=np.float32)
    weight = np.asarray(weight, dtype=np.float32)
    bias = np.asarray(bias, dtype=np.float32)
    subtract = np.asarray(subtract, dtype=np.float32)

    nc = _get_nc()

    xs = build_xstack(x)                      # [B, D, 128, PLANE]
    wr = build_wrhs(weight).astype(W_NP_DT)   # [128, 256] cols (c, parity)
    # biasrep/subrep: [128, 256] = (8 groups x 32 ch) pattern, bf16
    biasrep = np.tile(bias[None, None, :], (128, GRP, 1)).reshape(128, 256)
    biasrep = biasrep.astype(ml_dtypes.bfloat16)
    subrep = np.tile(subtract[None, None, :], (128, GRP, 1)).reshape(
        128, 256).astype(ml_dtypes.bfloat16)

    in_maps = []
    for c in range(N_CORES):
        in_maps.append({
            "xs": np.ascontiguousarray(xs[c * B_PER_CORE:(c + 1) * B_PER_CORE]),
            "wr": wr,
            "biasrep": biasrep,
            "subrep": subrep,
        })

    kw = {}
    if os.environ.get("KERNEL_TRACE_DIR"):
        kw["tmpdir"] = os.environ["KERNEL_TRACE_DIR"]
    res = run_bass_kernel_spmd(nc, in_maps, core_ids=list(range(N_CORES)), **kw)
    global LAST_EXEC_NS, LAST_RESULTS
    LAST_EXEC_NS = res.exec_time_ns
    LAST_RESULTS = res
    outs = [res.results[c]["y"].reshape(B_PER_CORE, D, H, W)
            for c in range(N_CORES)]
    return np.concatenate(outs, axis=0)
